# revision 2
# baseline (speedup 1.0000x reference)
"""Trainium2 Bass kernel for the ArcModel2Phase MC-integral loss — v3.

Key structural facts this version exploits (all verified on the axon
backend or in sim_numerics.py):

1. MC-sample sparsity: only ~49 of the 256 MC samples can contribute more
   than e^-30 of any column's sum (G(tx) > y_max + 0.6 makes 80% of the
   tx range dead for EVERY observation). The survivor set is computed
   exactly on the host P-grid; dropped mass <= 1e-10 relative. Survivors
   (padded to 64) live on TWO partition bands (0-63 and 64-127), so one
   [128, 512] PSUM tile holds TWO m-tiles of s'.

2. fp8e4(DoubleRow) matmuls: contraction depth is cost-free, so the f64
   operands are split into ~38 exact e4m3 rows; DoubleRow halves the
   per-output-column cost. A per-column baseline row (-P~ from a host
   grid) keeps s' <= ~6 so exp fits fp8e5.

3. exp via exponent-stuffing into fp8e5 bits, bit-identical on ACT
   (activation Copy w/ scale+bias -> u8) and DVE (tensor_scalar -> u8),
   with SH8 tuned for zero mean sawtooth. GPSIMD cannot read PSUM, so
   only ACT+DVE carry the elementwise stream: 16 groups of [128, 1024].

4. One fp8e5 DoubleRow reduce-matmul per 4-m-tile group: banded
   indicator weights route (band, col-half) -> 4 accumulator rows, so
   the full 64x512 column-sum tensor costs 16 x 107 ns on the PE and
   needs no pair-adds anywhere.

Host does O(N + M + grid) prep/post in f64 (same contract as the
baseline kernel: device does all O(N_eff * M) work).
"""

import math
from contextlib import ExitStack

import numpy as np
import ml_dtypes

import concourse.bass as bass
import concourse.tile as tile
from concourse import bacc, mybir
from concourse.bass_utils import run_bass_kernel_spmd

F32 = mybir.dt.float32
U8 = mybir.dt.uint8
F8E4 = mybir.dt.float8e4
F8E5 = mybir.dt.float8e5
AF = mybir.ActivationFunctionType
DR = mybir.MatmulPerfMode.DoubleRow
E4 = ml_dtypes.float8_e4m3
E5 = ml_dtypes.float8_e5m2

M = 262144
N_MC = 256
N_CORES = 8
MC = M // N_CORES            # 32768 observations per core
MT = 512                     # m-tile width (PSUM bank columns)
N_MTILES = MC // MT          # 64
N_GROUPS = N_MTILES // 4     # 16 groups of 4 m-tiles
NE_PAD = 64                  # survivor rows padded to one partition band
THETA = 30.0                 # survivor threshold (nats below global max)
CHUNK_MTS = [4, 8, 16, 16, 20]  # rhs DMA chunks (m-tiles, multiples of 4)
GROUP_LAG = 3                # groups between exp emission and its reduce
WIDTH_FACTOR = 2.5

# DoubleRow matmuls may only write PSUM base-partition 0 (hw ISA check
# s3d3_mm_valid_dst_partition), so band B (partitions 64-127) uses a plain
# fp8 matmul reading a flat (un-paired) copy of the same rows.

S_MARGIN = 6.0
SH8 = 0.183
SCH_A8 = float(np.float32(4.0 / math.log(2.0)))
SCH_B8 = float(np.float32(60.0 - SH8))

# fp8 split-product pairs (term i of left factor x term j of right factor)
PAIRS_TDX = [(i, j) for i in range(4) for j in range(4) if i + j <= 3]
PAIRS_GDY = [(i, j) for i in range(4) for j in range(4) if i + j <= 4]
NT_A = 5                     # e4m3 terms for the A (per-MC bias) rows
NT_V = 5                     # e4m3 terms for the V (per-m baseline) rows
N_ROWS_RAW = len(PAIRS_TDX) + len(PAIRS_GDY) + NT_A + NT_V
K2P = (N_ROWS_RAW + 1) // 2          # DoubleRow partitions
KFL = 2 * K2P                        # flat contraction rows (plain matmul)
HEAD_COLS = 2 * NE_PAD + NE_PAD      # DR lhsT block + flat lhsT block

COST_ACT_GRP = 1038.0        # one [128,1024] activation(Copy) op
COST_DVE_GRP = 1316.0        # two [128,512] tensor_scalar ops
IND_Z = 28                   # indicator role-column base in the bank


def _erfinv(u):
    try:
        from scipy.special import erfinv as sp_erfinv
        return np.asarray(sp_erfinv(u), dtype=np.float64)
    except Exception:
        u = np.asarray(u, dtype=np.float64)
        aa = 0.147
        ln1mu2 = np.log1p(-u * u)
        term = 2.0 / (np.pi * aa) + ln1mu2 / 2.0
        w = np.sign(u) * np.sqrt(np.sqrt(term * term - ln1mu2 / aa) - term)
        erf_v = np.vectorize(math.erf)
        c = 2.0 / math.sqrt(math.pi)
        for _ in range(4):
            w = w - (erf_v(w) - u) / (c * np.exp(-w * w))
        return w


def _make_schedule():
    """Greedy group assignment to ACT ('A') or DVE ('D'); the final acc
    copy is pre-charged to DVE and the last group forced to ACT so DVE is
    free the moment the last reduce lands."""
    return list("ADADAADADADAADAA")


SCHEDULE = _make_schedule()


def _build_graph():
    nc = bacc.Bacc("TRN2", target_bir_lowering=False, debug=False,
                   num_devices=N_CORES)
    rhs_ext = nc.declare_dram_parameter("rhs", [KFL, HEAD_COLS + 3 * MC], U8,
                                        isOutput=False)
    out_ext = nc.declare_dram_parameter("out", [N_MTILES, MT], F32, isOutput=True)

    with tile.TileContext(nc) as tc:
        with ExitStack() as ctx:
            singles = ctx.enter_context(tc.tile_pool(name="singles", bufs=1))
            exp_pool = ctx.enter_context(tc.tile_pool(name="exp", bufs=4))
            psA = ctx.enter_context(tc.tile_pool(name="psA", bufs=2, space="PSUM"))
            psD = ctx.enter_context(tc.tile_pool(name="psD", bufs=2, space="PSUM"))
            acc_pool = ctx.enter_context(tc.tile_pool(name="acc", bufs=1, space="PSUM"))
            acc2_pool = ctx.enter_context(tc.tile_pool(name="acc2", bufs=1, space="PSUM"))

            # chunk 0: single fused DMA [KFL, heads + DR0 + flat0]; later
            # chunks: two lean DMAs each (DR rows only / flat rows only)
            rhs_tiles = []
            mt0 = 0
            col0 = 0
            lhsT3 = None
            lhsT_fl = None
            for q, nmt in enumerate(CHUNK_MTS):
                cw = nmt * MT
                if q == 0:
                    w = HEAD_COLS + 3 * cw
                    rt = singles.tile([KFL, w], U8, name="rhs0")
                    nc.sync.dma_start(out=rt[:],
                                      in_=rhs_ext.ap()[:, col0:col0 + w])
                    lhsT3 = (rt[0:K2P, 0:2 * NE_PAD].bitcast(F8E4)
                             .rearrange("p (two f) -> p two f", two=2))
                    lhsT_fl = rt[:, 2 * NE_PAD:HEAD_COLS].bitcast(F8E4)
                    dr_ap = (rt[0:K2P, HEAD_COLS:HEAD_COLS + 2 * cw]
                             .bitcast(F8E4)
                             .rearrange("p (two n) -> p two n", two=2))
                    fl_ap = rt[:, HEAD_COLS + 2 * cw:w].bitcast(F8E4)
                else:
                    w = 3 * cw
                    rtd = singles.tile([K2P, 2 * cw], U8, name=f"rhsd{q}")
                    nc.sync.dma_start(
                        out=rtd[:],
                        in_=rhs_ext.ap()[0:K2P, col0:col0 + 2 * cw])
                    rtf = singles.tile([KFL, cw], U8, name=f"rhsf{q}")
                    nc.scalar.dma_start(
                        out=rtf[:],
                        in_=rhs_ext.ap()[:, col0 + 2 * cw:col0 + w])
                    dr_ap = (rtd[:].bitcast(F8E4)
                             .rearrange("p (two n) -> p two n", two=2))
                    fl_ap = rtf[:].bitcast(F8E4)
                rhs_tiles.append((mt0, nmt, dr_ap, fl_ap))
                mt0 += nmt
                col0 += w

            def rhs_slice(mt):
                """(DR-paired AP, flat AP) for m-tile mt."""
                for m0, nmt, ap3, apf in rhs_tiles:
                    if m0 <= mt < m0 + nmt:
                        loc = mt - m0
                        return (ap3[:, :, loc * MT:(loc + 1) * MT],
                                apf[:, loc * MT:(loc + 1) * MT])
                raise AssertionError(mt)

            # banded indicator bank [128, 2, 64] fp8e5:
            #   (band0, j=0) -> role col Z   ... (band1, j=1) -> Z+3
            ind_sb = singles.tile([128, 128], U8)
            ind3 = ind_sb[:].bitcast(F8E5).rearrange("p (two w) -> p two w", two=2)
            nc.gpsimd.memset(ind_sb[:], 0)
            nc.gpsimd.memset(ind3[0:NE_PAD, 0:1, IND_Z:IND_Z + 1], 1.0)
            nc.gpsimd.memset(ind3[NE_PAD:128, 0:1, IND_Z + 1:IND_Z + 2], 1.0)
            nc.gpsimd.memset(ind3[0:NE_PAD, 1:2, IND_Z + 2:IND_Z + 3], 1.0)
            nc.gpsimd.memset(ind3[NE_PAD:128, 1:2, IND_Z + 3:IND_Z + 4], 1.0)

            acc0 = acc_pool.tile([32, MT], F32, name="acc0")
            acc1 = acc2_pool.tile([32, MT], F32, name="acc1")

            # PE warm-up: dummy matmuls over the (zero) indicator bank keep
            # the tensor engine busy through the initial DMA window so the
            # cost-model p-state is warm when real work arrives. They write
            # acc1, which the first real reduce (start=True) re-zeroes.
            for _ in range(40):
                nc.tensor.matmul(acc1[0:32, 0:64], ind3[:, :, 0:32],
                                 ind3[:, :, 0:64], start=True, stop=True,
                                 perf_mode=DR, skip_group_check=True)
            pend_reduce = []

            def emit_reduce(g, ex):
                tgt = acc0 if g < 8 else acc1
                q = g % 8
                ex3 = ex[:].bitcast(F8E5).rearrange("p (two n) -> p two n", two=2)
                nc.tensor.matmul(tgt[:], ind3[:, :, IND_Z - 4 * q:IND_Z - 4 * q + 32],
                                 ex3, start=(q == 0), stop=(q == 7),
                                 perf_mode=DR)
                if g == 7:
                    res = singles.tile([32, MT], F32, name="res0")
                    nc.vector.tensor_copy(out=res[:], in_=tgt[:])
                    nc.sync.dma_start(out=out_ext.ap()[0:32, :], in_=res[:])
                elif g == N_GROUPS - 2:
                    # rows 0..27 of acc1 are final once group 14 reduces
                    res = singles.tile([28, MT], F32, name="res1")
                    nc.vector.tensor_copy(out=res[:], in_=tgt[0:28, :])
                    nc.sync.dma_start(out=out_ext.ap()[32:60, :], in_=res[:])
                elif g == N_GROUPS - 1:
                    res = singles.tile([32, MT], F32, name="res2")
                    nc.vector.tensor_copy(out=res[:], in_=tgt[:])
                    nc.sync.dma_start(out=out_ext.ap()[60:64, :],
                                      in_=res[28:32, :])

            def emit_mm(pt_slice, mt, band):
                rdr, rfl = rhs_slice(mt)
                if band == 0:
                    nc.tensor.matmul(pt_slice, lhsT3, rdr,
                                     start=True, stop=True, perf_mode=DR)
                else:
                    nc.tensor.matmul(pt_slice, lhsT_fl, rfl,
                                     start=True, stop=True)

            for g in range(N_GROUPS):
                ex = exp_pool.tile([128, 1024], U8, name="ex", tag="ex")
                if SCHEDULE[g] == "A":
                    pt = psA.tile([128, 1024], F32, name="pa", tag="pa")
                    for i in range(4):
                        b = i % 2
                        emit_mm(pt[b * NE_PAD:(b + 1) * NE_PAD,
                                   (i // 2) * MT:(i // 2) * MT + MT],
                                4 * g + i, b)
                    nc.scalar.activation(out=ex[:, 0:1024], in_=pt[:],
                                         func=AF.Copy,
                                         bias=SCH_B8, scale=SCH_A8)
                else:
                    for half in range(2):
                        pt = psD.tile([128, MT], F32, name="pd", tag="pd")
                        for i in range(2):
                            emit_mm(pt[i * NE_PAD:(i + 1) * NE_PAD, :],
                                    4 * g + 2 * half + i, i)
                        nc.vector.tensor_scalar(
                            out=ex[:, half * MT:(half + 1) * MT], in0=pt[:],
                            scalar1=SCH_A8, scalar2=SCH_B8,
                            op0=mybir.AluOpType.mult, op1=mybir.AluOpType.add)
                pend_reduce.append((g, ex))
                if len(pend_reduce) > GROUP_LAG:
                    emit_reduce(*pend_reduce.pop(0))
            while pend_reduce:
                emit_reduce(*pend_reduce.pop(0))

    nc.compile()
    return nc


_GRAPH = None


def _get_graph():
    global _GRAPH
    if _GRAPH is None:
        _GRAPH = _build_graph()
    return _GRAPH


def _prescale(v):
    mx = np.abs(v).max()
    if mx == 0:
        return 1.0
    return 2.0 ** math.floor(math.log2(120.0 / mx))


def _split_fp8(v, nterms):
    v = np.asarray(v, np.float64)
    terms = []
    rem = v.copy()
    for _ in range(nterms):
        h = np.clip(rem, -240, 240).astype(E4).astype(np.float64)
        terms.append(h)
        rem = rem - h
    return terms


def _prepare_inputs(x, y, k_u, sigma_b, sigma_n, I1, I2, w1, w2, w12):
    x = np.asarray(x, dtype=np.float64)
    y = np.asarray(y, dtype=np.float64)
    k_u = np.asarray(k_u, dtype=np.float64)
    assert x.shape == (M,) and y.shape == (M,) and k_u.shape == (N_MC,), (
        f"kernel compiled for M={M}, N_MC={N_MC}; got {x.shape} {y.shape} {k_u.shape}")
    sigma_b = float(np.asarray(sigma_b))
    sigma_n = float(np.asarray(sigma_n))
    I1 = float(np.asarray(I1)); I2 = float(np.asarray(I2))
    w1 = float(np.asarray(w1).reshape(-1)[0])
    w2 = float(np.asarray(w2).reshape(-1)[0])
    w12 = float(np.asarray(w12).reshape(-1)[0])

    sn2 = sigma_n * sigma_n
    LOG2PI = math.log(2.0 * math.pi)
    Wf = WIDTH_FACTOR

    r = np.array([w1, w2, w12])
    rmax = r.max()
    lw = r - (rmax + math.log(np.exp(r - rmax).sum()))

    I_min = I1 + 0.5 * (I2 - I1) * (1.0 + math.erf(-Wf / math.sqrt(2.0)))
    I_diff = (I2 - I1) * math.erf(Wf / math.sqrt(2.0))
    tx = k_u * I_diff + I_min
    u = 2.0 * (tx - I1) / (I2 - I1) - 1.0
    ei = _erfinv(u)
    G = (I2 - I1) / math.sqrt(2.0 * math.pi * sigma_b ** 2) * np.exp(-ei ** 2)
    t = tx / sn2
    g = 2.0 * G / sn2
    a = -np.log(G) - G ** 2 / sn2 - tx ** 2 / (2.0 * sn2) + ei ** 2
    K_const = (-math.log(sigma_n) - 0.5 * LOG2PI
               + math.log(2.0) - 2.0 * math.log(sigma_n)
               + 0.5 * math.log(2.0 / math.pi) - 0.5 * math.log(2.0)
               + math.log(sigma_n) - math.log(2.0)
               - math.log(2.0 * Wf * (I2 - I1)) + 0.5 * LOG2PI)

    x0 = 0.5 * (x.min() + x.max())
    y0 = 0.5 * (y.min() + y.max())
    dx = x - x0
    dy = y - y0
    A = a + t * x0 + g * y0
    b = np.log(y) - y ** 2 / sn2 - x ** 2 / (2.0 * sn2)

    # P grid (max over ALL samples) + survivor margins
    NGX, NGY = 256, 1024
    gx = np.linspace(dx.min(), dx.max(), NGX)
    gy = np.linspace(dy.min(), dy.max(), NGY)
    P = np.full((NGX, NGY), -np.inf)
    margin = np.empty(N_MC)
    phis = []
    for i0 in range(0, N_MC, 32):
        sl = slice(i0, i0 + 32)
        phi = (A[sl, None, None] + t[sl, None, None] * gx[None, :, None]
               + g[sl, None, None] * gy[None, None, :])
        phis.append(phi)
        P = np.maximum(P, phi.max(axis=0))
    for i0 in range(0, N_MC, 32):
        phi = phis[i0 // 32]
        margin[i0:i0 + 32] = (phi - P[None]).reshape(32, -1).max(axis=1)
    del phis
    sel = np.flatnonzero(margin > -THETA)
    assert len(sel) <= NE_PAD, (
        f"survivor count {len(sel)} exceeds {NE_PAD}; input distribution "
        "differs from the compiled assumption")

    fx = (dx - gx[0]) / (gx[1] - gx[0])
    fy = (dy - gy[0]) / (gy[1] - gy[0])
    ix = np.clip(fx.astype(int), 0, NGX - 2)
    iy = np.clip(fy.astype(int), 0, NGY - 2)
    wx = fx - ix
    wy = fy - iy
    Pt = (P[ix, iy] * (1 - wx) * (1 - wy) + P[ix + 1, iy] * wx * (1 - wy)
          + P[ix, iy + 1] * (1 - wx) * wy + P[ix + 1, iy + 1] * wx * wy)

    V = S_MARGIN - Pt
    Ct = b + Pt - S_MARGIN

    ts_, gs_, As_ = t[sel], g[sel], A[sel]
    ne = len(sel)

    rows = []
    st, sdx = _prescale(ts_), _prescale(dx)
    sg_, sdy = _prescale(gs_), _prescale(dy)
    sA, sV = _prescale(As_), _prescale(V)
    tT = _split_fp8(ts_ * st, 4)
    dxT = _split_fp8(dx * sdx, 4)
    gT = _split_fp8(gs_ * sg_, 4)
    dyT = _split_fp8(dy * sdy, 4)
    AT = _split_fp8(As_ * sA, NT_A)
    VT = _split_fp8(V * sV, NT_V)
    onesN = np.ones(ne)
    onesM = np.ones(M)
    for i, j in PAIRS_TDX:
        rows.append((tT[i] / st, dxT[j] / sdx))
    for i, j in PAIRS_GDY:
        rows.append((gT[i] / sg_, dyT[j] / sdy))
    for at in AT:
        rows.append((at / sA, onesM))
    for vt in VT:
        rows.append((onesN, vt / sV))
    while len(rows) < 2 * K2P:
        rows.append((np.zeros(ne), np.zeros(M)))

    LH = np.zeros((2 * K2P, NE_PAD), dtype=E4)   # pad cols stay 0
    RH = np.empty((2 * K2P, M), dtype=E4)
    for k, (lv, rv) in enumerate(rows):
        lmax = np.abs(lv).max(); rmax_ = np.abs(rv).max()
        lam = 1.0
        if lmax > 0 and rmax_ > 0:
            lam = 2.0 ** round(0.5 * (math.log2(rmax_) - math.log2(lmax)))
        LH[k, :ne] = np.clip(lv * lam, -240, 240).astype(E4)
        RH[k] = np.clip(rv / lam, -240, 240).astype(E4)

    # heads: DR-paired lhsT on rows 0..K2P, flat lhsT on all KFL rows
    lh_bits = LH.view(np.uint8)                       # [KFL, NE_PAD]
    head = np.zeros((KFL, HEAD_COLS), np.uint8)
    head[:K2P, :2 * NE_PAD] = lh_bits.reshape(K2P, 2 * NE_PAD)
    head[:, 2 * NE_PAD:] = lh_bits
    rhs_fl = RH.view(np.uint8)                        # [KFL, M]
    rhs_dr = rhs_fl.reshape(K2P, 2, M)

    in_maps = []
    for c in range(N_CORES):
        parts = [head]
        mt0 = 0
        for nmt in CHUNK_MTS:
            cw = nmt * MT
            sl = slice(c * MC + mt0 * MT, c * MC + mt0 * MT + cw)
            blk = np.zeros((KFL, 3 * cw), np.uint8)
            blk[:K2P, :2 * cw] = rhs_dr[:, :, sl].reshape(K2P, 2 * cw)
            blk[:, 2 * cw:] = rhs_fl[:, sl]
            parts.append(blk)
            mt0 += nmt
        in_maps.append({"rhs": np.concatenate(parts, axis=1)})

    D = lw[2] + K_const + math.log(I_diff) - math.log(N_MC)
    C2 = (math.log(2.0) - math.lgamma(1.5) - 4.0 * math.log(sigma_n)
          - 0.5 * LOG2PI)
    lp1 = C2 + 2.0 * np.log(y) - (y / sigma_n) ** 2 - 0.5 * ((x - I1) / sigma_n) ** 2 + lw[0]
    lp2 = C2 + 2.0 * np.log(y) - (y / sigma_n) ** 2 - 0.5 * ((x - I2) / sigma_n) ** 2 + lw[1]
    uu = np.logaddexp(lp1, lp2)
    return in_maps, D, Ct, uu


def _combine(results, D, Ct, uu):
    colsum = np.concatenate(
        [results[c]["out"].astype(np.float64).reshape(MC) for c in range(N_CORES)])
    lp12 = D + Ct + np.log(np.maximum(colsum, 1e-300))
    mx = np.maximum(uu, lp12)
    loss = -(mx + np.log(np.exp(uu - mx) + np.exp(lp12 - mx))).sum()
    return np.float32(loss)


def kernel(x, y, k_u, sigma_b, sigma_n, I1, I2, w1, w2, w12):
    nc = _get_graph()
    in_maps, D, Ct, uu = _prepare_inputs(x, y, k_u, sigma_b, sigma_n, I1, I2,
                                         w1, w2, w12)
    res = run_bass_kernel_spmd(nc, in_maps, core_ids=list(range(N_CORES)))
    return _combine(res.results, D, Ct, uu)


def run_traced(x, y, k_u, sigma_b, sigma_n, I1, I2, w1, w2, w12, **kw):
    nc = _get_graph()
    in_maps, D, Ct, uu = _prepare_inputs(x, y, k_u, sigma_b, sigma_n, I1, I2,
                                         w1, w2, w12)
    res = run_bass_kernel_spmd(nc, in_maps, core_ids=list(range(N_CORES)),
                               trace=True, **kw)
    return _combine(res.results, D, Ct, uu), res
